# revision 29
# baseline (speedup 1.0000x reference)
"""Trainium2 Bass kernel for nn_Attention_42253888258536.

Full-precision (fp32) multi-head attention with RoPE:
  qkv = x @ qkv_w.T + qkv_b ; RoPE(q, k) ; softmax(q k^T / sqrt(hd)) @ v ; proj.

Sharding: 8 cores = 2 batches x 4 head-groups (2 heads each). Each core
computes its heads' attention and a partial output projection (row-parallel
over proj_w columns); the host sums 4 partials per batch and adds proj_b.

Per-core device pipeline (all fp32 accumulation):
  1. q^T/k^T = W @ x^T via PE (weights stationary), v in natural layout.
  2. RoPE in transposed layout: rotate_half as a permutation matmul on PE,
     combine with cos/sin tables on DVE (f16 temporaries for 2x DVE modes).
  3. Attention over S^T = k_rot q_rot^T tiles: the two heads' K=64 matmuls
     run concurrently as PE row-tiles; exp on ACT (scale=1/8, bias=-5 fused).
     The ACT engine is the roofline for this kernel (1 elem/lane/cycle
     @1.2GHz over B*H*L^2/8 = 33.5M scores per core), so a tunable fraction
     of the exp tiles is offloaded to the otherwise-idle DVE using a
     Schraudolph-style bit trick: bits16 = (int16)(A*s + B) reinterpreted
     as bfloat16 gives exp(s/8-5) to ~1.8% rel, which softmax normalization
     largely cancels (<=1.3e-2 end-to-end vs the 2e-2 gate).
  4. P@V accumulated in PSUM with a ones-row appended to V so the softmax
     denominator Z falls out of the same matmul (row 64 of the ctx copy).
  5. Deferred normalization: out_h = (ctx_h @ Wp_h^T) * (1/Z) per partition,
     heads combined on DVE, partial written to DRAM.
"""

import sys

sys.path.insert(0, "/opt/trn_rl_repo")

import numpy as np

B, L, C = 2, 4096, 512
H, HD = 8, 64
NCORES = 8
HPC = 2          # heads per core
GROUPS = 4       # head groups (cores per batch)
QB = 512         # q-block (columns per S^T matmul)
NQB = L // QB    # 8
KT = 128         # k-tile (partitions per S^T tile)
NKT = L // KT    # 32

# DVE exp-offload (Schraudolph bits-as-bf16). OFF_MOD=0 disables.
OFF_MOD = 4
LOG2E = 1.4426950408889634
SCH_A = 128.0 * LOG2E / 8.0                       # includes the 1/8 score scale
SCH_SIGMA = 7.0                                   # mean-centering offset
SCH_B = 128.0 * (127.0 - 5.0 * LOG2E) - SCH_SIGMA

_NC_CACHE = {}


def _offloaded(qb, kt):
    # kt % 4 == 1 avoids the q_chain (kt==2) and proj (kt==6) DVE bursts;
    # qb0's early kts are skipped (DVE busy with k/v chains there).
    return OFF_MOD > 0 and (qb >= 1 or kt >= 8) and kt % OFF_MOD == 1


def _emit(tc, nc, ins, out_ap, mybir, bass):
    f32 = mybir.dt.float32
    f16 = mybir.dt.float16           # full-rate PE dtype that also keeps the HAM clock gate warm
    bf16 = mybir.dt.bfloat16
    i16 = mybir.dt.int16
    Exp = mybir.ActivationFunctionType.Exp
    Alu = mybir.AluOpType

    xT, wqkT, wvT, qkb, vb, cos2, sin2, prhT, wpT = (
        ins["xT"], ins["wqkT"], ins["wvT"], ins["qkb"], ins["vb"],
        ins["cos2"], ins["sin2"], ins["prhT"], ins["wpT"],
    )

    with tc.tile_pool(name="const", bufs=1) as const:
        xT_sb = const.tile([128, 4, L], f16)
        wqk_sb = const.tile([128, 4, 2 * HPC * HD], f16)
        wv_sb = const.tile([128, 4, HPC * HD], f16)
        qkb_sb = const.tile([128, 2], f32)
        vb_sb = const.tile([128, HPC * HD], f32)
        cos_sb = const.tile([128, L], f16)
        sin_sb = const.tile([128, L], f16)
        prh_sb = const.tile([128, 128], f16)
        wp_sb = const.tile([128, C], f16)
        expbias = const.tile([128, 1], f32)
        actwarm = const.tile([128, 1], f32)
        nc.vector.memset(expbias[:], -5.0)

        # DMA priority order: stream inputs in the order the qb=0 pipeline
        # consumes them (block 0 weights/x first, then x block-by-block with
        # its cos/sin). The scalar queue only carries kickoffs that complete
        # before the first ACTIVATE; everything else is sync/gpsimd (the ACT
        # engine is the roofline and every queued kickoff costs ~620ns).
        # Only the sync and scalar queues feed fast HWDGE rings (~90 GB/s
        # each); the gpsimd SWDGE ring is ~17 GB/s, so it carries no bulk
        # input. k_chain(0) consumes (xT b0 cc, wqk cc) in cc order:
        # interleave so its accumulation starts as soon as chunks land.
        for cc in range(4):
            eng = nc.sync if cc % 2 == 0 else nc.scalar
            eng.dma_start(xT_sb[:, cc, 0:QB], xT[cc * 128:(cc + 1) * 128, 0:QB])
            eng.dma_start(wqk_sb[:, cc, :], wqkT[cc * 128:(cc + 1) * 128, :])
        nc.sync.dma_start(qkb_sb[:], qkb[:])
        nc.scalar.dma_start(prh_sb[:], prhT[:])
        nc.sync.dma_start(cos_sb[:, 0:QB], cos2[:, 0:QB])
        nc.scalar.dma_start(sin_sb[:, 0:QB], sin2[:, 0:QB])
        # x block 1 plus its cos/sin (q_chain(1) fires at qb0/kt==2)
        for cc in range(4):
            eng = nc.sync if cc % 2 == 0 else nc.scalar
            eng.dma_start(xT_sb[:, cc, QB:2 * QB], xT[cc * 128:(cc + 1) * 128, QB:2 * QB])
        nc.sync.dma_start(cos_sb[:, QB:2 * QB], cos2[:, QB:2 * QB])
        nc.scalar.dma_start(sin_sb[:, QB:2 * QB], sin2[:, QB:2 * QB])
        # dummy exp: forces the ~2.7us exp ACT_TABLE_LOAD to overlap the
        # input DMAs instead of stalling the first real exp
        nc.scalar.activation(actwarm[:], expbias[:], Exp, bias=expbias[:], scale=1.0)
        for cc in range(4):
            nc.sync.dma_start(wv_sb[:, cc, :], wvT[cc * 128:(cc + 1) * 128, :])
        nc.sync.dma_start(vb_sb[:], vb[:])
        # x blocks 2-7 in two 1536-col waves per channel group; the sync ring
        # takes cc0/2 + cos, the scalar ring's share is kicked off from inside
        # the qb0 loop (between early exps) to stay behind the warm activation.
        for wv_ in range(2):
            wsl = slice((2 + 3 * wv_) * QB, (5 + 3 * wv_) * QB)
            for cc in (0, 2):
                nc.sync.dma_start(xT_sb[:, cc, wsl], xT[cc * 128:(cc + 1) * 128, wsl])
            nc.sync.dma_start(cos_sb[:, wsl], cos2[:, wsl])
        nc.sync.dma_start(wp_sb[:], wpT[:])

        def late_input_wave(wv_):
            wsl = slice((2 + 3 * wv_) * QB, (5 + 3 * wv_) * QB)
            for cc in (1, 3):
                nc.scalar.dma_start(xT_sb[:, cc, wsl], xT[cc * 128:(cc + 1) * 128, wsl])
            nc.scalar.dma_start(sin_sb[:, wsl], sin2[:, wsl])

        with tc.tile_pool(name="work", bufs=1) as work:
            qT_sb = work.tile([128, L], f16)   # 2 heads x 64 dims on partitions
            kT_sb = work.tile([128, L], f16)
            # v_aug[:, kt, 65*h : 65*h+65] = [V_h | ones] for k-tile kt
            v_aug = work.tile([128, NKT, 2 * (HD + 1)], f16)
            # ctxB rows 0-63 head0 ctx, row 64 holds Z0 until the zc gather,
            # rows 64-127 head1 ctx after the ctx1s DMA; ctx1s row 64 = Z1.
            ctxB = work.tile([128, L], f16)
            ctx1s = work.tile([HD + 1, L], f16)

            nc.vector.memset(v_aug[:, :, HD:HD + 1], 1.0)
            nc.vector.memset(v_aug[:, :, 2 * HD + 1:2 * HD + 2], 1.0)

            # ---- attention with fused v/q/proj pipelines ----
            with tc.tile_pool(name="spsum", bufs=2, space="PSUM") as spsum, \
                 tc.tile_pool(name="pv0ps", bufs=1, space="PSUM") as pv0ps, \
                 tc.tile_pool(name="pv1ps", bufs=1, space="PSUM") as pv1ps, \
                 tc.tile_pool(name="auxps", bufs=1, space="PSUM") as auxps, \
                 tc.tile_pool(name="psb", bufs=7) as psb, \
                 tc.tile_pool(name="psb16", bufs=5) as psb16, \
                 tc.tile_pool(name="auxsb", bufs=4) as auxsb, \
                 tc.tile_pool(name="zsb", bufs=10) as zsb, \
                 tc.tile_pool(name="outsb", bufs=3) as outsb:

                def k_chain(lb):
                    # k projection + RoPE for one 512-block on the aux banks;
                    # blocks 1-7 are emitted inside qb=0's kt loop so attention
                    # starts as soon as k-block 0 is roped.
                    lsl = bass.ts(lb, QB)
                    ps = auxps.tile([128, QB], f32, tag="aux0", name="kps")[:]
                    for cc in range(4):
                        nc.tensor.matmul(ps, wqk_sb[:, cc, 128:256], xT_sb[:, cc, lsl],
                                         start=(cc == 0), stop=(cc == 3))
                    nc.vector.tensor_scalar_add(kT_sb[:, lsl], ps, qkb_sb[:, 1:2])
                    rh = auxps.tile([128, QB], f32, tag="aux1", name="krh")[:]
                    nc.tensor.matmul(rh, prh_sb[:], kT_sb[:, lsl], start=True, stop=True)
                    t1 = auxsb.tile([128, QB], f16, tag="qt1")
                    nc.vector.tensor_mul(t1[:], kT_sb[:, lsl], cos_sb[:, lsl])
                    t2 = auxsb.tile([128, QB], f16, tag="qt2")
                    nc.vector.tensor_mul(t2[:], rh, sin_sb[:, lsl])
                    nc.vector.tensor_add(kT_sb[:, lsl], t1[:], t2[:])

                def q_chain(lb):
                    # q projection + RoPE for one 512-block, time-sharing the
                    # two aux PSUM banks with the projection pipeline.
                    lsl = bass.ts(lb, QB)
                    ps = auxps.tile([128, QB], f32, tag="aux0", name="qps")[:]
                    for cc in range(4):
                        nc.tensor.matmul(ps, wqk_sb[:, cc, 0:128], xT_sb[:, cc, lsl],
                                         start=(cc == 0), stop=(cc == 3))
                    nc.vector.tensor_scalar_add(qT_sb[:, lsl], ps, qkb_sb[:, 0:1])
                    rh = auxps.tile([128, QB], f32, tag="aux1", name="qrh")[:]
                    nc.tensor.matmul(rh, prh_sb[:], qT_sb[:, lsl], start=True, stop=True)
                    t1 = auxsb.tile([128, QB], f16, tag="qt1")
                    nc.vector.tensor_mul(t1[:], qT_sb[:, lsl], cos_sb[:, lsl])
                    t2 = auxsb.tile([128, QB], f16, tag="qt2")
                    nc.vector.tensor_mul(t2[:], rh, sin_sb[:, lsl])
                    nc.vector.tensor_add(qT_sb[:, lsl], t1[:], t2[:])

                def v_chain(lt):
                    ps = auxps.tile([128, 128], f32, tag="aux1", name="vps")[:]
                    for cc in range(4):
                        nc.tensor.matmul(ps, xT_sb[:, cc, bass.ts(lt, 128)], wv_sb[:, cc, :],
                                         start=(cc == 0), stop=(cc == 3))
                    nc.vector.tensor_tensor(
                        v_aug[:, lt, :].rearrange("p (h x) -> p h x", h=2)[:, :, 0:HD],
                        ps.rearrange("p (h x) -> p h x", h=2),
                        vb_sb[:].rearrange("p (h x) -> p h x", h=2),
                        op=Alu.add,
                    )

                z_tiles = {}
                pv_state = {}
                p_tiles = {}

                def proj_block(qb, tail=False):
                    # projection + 1/Z + head-combine + output DMA for q-block qb.
                    # In the tail (last qb), the scalar queue is idle, so the
                    # head1 scaling moves to ACT and the serialized DVE chain
                    # roughly halves.
                    zt = z_tiles.pop(qb)
                    for j in range(QB // 128):
                        qi = qb * (QB // 128) + j
                        qisl = bass.ts(qi, 128)
                        zc = zsb.tile([128, 2], f32, tag="zc")
                        nc.vector.reciprocal(zc[:], zt[:, :, j])
                        p0 = auxps.tile([128, C], f32, tag="aux0", name="p0")
                        p1 = auxps.tile([128, C], f32, tag="aux1", name="p1")
                        nc.tensor.matmul(p0[:], ctxB[0:HD, qisl], wp_sb[0:HD, :],
                                         start=True, stop=True)
                        nc.tensor.matmul(p1[:], ctxB[HD:128, qisl], wp_sb[HD:128, :],
                                         start=True, stop=True)
                        tmp = auxsb.tile([128, C], f32, tag="tmp")
                        # ACT Copy-with-scale: flexible filler the scheduler can
                        # slot into exp-ring bubbles; also relieves the DVE
                        nc.scalar.activation(tmp[:], p1[:],
                                             mybir.ActivationFunctionType.Copy,
                                             scale=zc[:, 1:2])
                        ot = outsb.tile([128, C], f32, tag="ot")
                        nc.vector.scalar_tensor_tensor(
                            ot[:], p0[:], zc[:, 0:1], tmp[:],
                            op0=Alu.mult, op1=Alu.add,
                        )
                        eng = nc.scalar if tail and j % 2 else nc.sync
                        eng.dma_start(out_ap[qisl, :], ot[:])

                def pv_step(qb, kt):
                    pv0, pv1 = pv_state[qb]
                    off, pk = p_tiles.pop(qb * NKT + kt)
                    r0 = pk[:, 0, :].bitcast(bf16) if off else pk[:, 0, :]
                    r1 = pk[:, 1, :].bitcast(bf16) if off else pk[:, 1, :]
                    nc.tensor.matmul(pv0[:], v_aug[:, kt, 0:HD + 1], r0,
                                     start=(kt == 0), stop=(kt == NKT - 1),
                                     skip_group_check=True)
                    nc.tensor.matmul(pv1[:], v_aug[:, kt, HD + 1:2 * (HD + 1)],
                                     r1,
                                     start=(kt == 0), stop=(kt == NKT - 1),
                                     skip_group_check=True)

                def qb_finish(qb):
                    # ctx copies (z rides in row 64), batched z gathers, then the
                    # head1 ctx DMA (which overwrites ctxB row 64 -- gathers first).
                    qsl = bass.ts(qb, QB)
                    pv0, pv1 = pv_state.pop(qb)
                    nc.vector.tensor_copy(ctxB[0:HD + 1, qsl], pv0[0:HD + 1, :])
                    nc.vector.tensor_copy(ctx1s[:, qsl], pv1[0:HD + 1, :])
                    zt = zsb.tile([128, 2, 4], f16, tag="zt")
                    for j in range(QB // 128):
                        qisl = bass.ts(qb * (QB // 128) + j, 128)
                        geng = nc.sync if j % 2 == 0 else \
                            (nc.scalar if qb == NQB - 1 else nc.sync)
                        geng.dma_start(zt[:, 0, j:j + 1], ctxB[HD:HD + 1, qisl])
                        geng.dma_start(zt[:, 1, j:j + 1], ctx1s[HD:HD + 1, qisl])
                    z_tiles[qb] = zt
                    nc.sync.dma_start(ctxB[HD:128, qsl], ctx1s[0:HD, qsl])

                # only k-block 0 and q-block 0 gate the first S matmul;
                # the remaining k/v-chains stream in during the first kt's,
                # just-in-time so a chain stalled on an input DMA never
                # head-of-line blocks a ready S matmul on the PE queue.
                # The PV/proj consumption runs 3 steps behind the S/exp stream
                # ACROSS qb boundaries, so the next block's first S never waits
                # behind the previous block's PV tail on the PE queue.
                k_chain(0)
                q_chain(0)
                v_chain(0)
                v_chain(1)
                TOT = NQB * NKT
                for g in range(TOT + 3):
                    if g < TOT:
                        qb, kt = divmod(g, NKT)
                        qsl = bass.ts(qb, QB)
                        if kt == 0:
                            pv_state[qb] = (
                                pv0ps.tile([HD + 1, QB], f32, tag="pv0", name="pv0"),
                                pv1ps.tile([HD + 1, QB], f32, tag="pv1", name="pv1"),
                            )
                        ksl = bass.ts(kt, KT)
                        s = spsum.tile([128, 2, QB], f32, tag="s")
                        nc.tensor.matmul(s[:, 0, :], kT_sb[0:HD, ksl],
                                         qT_sb[0:HD, qsl], start=True, stop=True)
                        nc.tensor.matmul(s[:, 1, :], kT_sb[HD:128, ksl],
                                         qT_sb[HD:128, qsl], start=True, stop=True)
                        if _offloaded(qb, kt):
                            # Schraudolph: (int16)(A*s + B) bits == bf16 exp(s/8-5).
                            # High priority: this op gates the 2-deep S-tile ring,
                            # so it must never queue behind proj/RoPE DVE bursts.
                            p = psb16.tile([128, 2, QB], i16, tag="poff", name="poff")
                            with tc.high_priority():
                                nc.vector.tensor_scalar(
                                    p[:], s[:], SCH_A, SCH_B,
                                    op0=Alu.mult, op1=Alu.add,
                                )
                            p_tiles[g] = (True, p)
                        else:
                            # exp(s/8 - 5): the shift keeps the f16 exp output far
                            # from overflow; softmax is shift-invariant since Z
                            # accumulates the same e^-5.
                            p = psb.tile([128, 2, QB], f16, tag="p", name="p")
                            nc.scalar.activation(p[:], s[:], Exp, bias=expbias[:],
                                                 scale=0.125)
                            p_tiles[g] = (False, p)
                        if qb == 0:
                            if kt == 0:
                                late_input_wave(0)
                            elif kt == 2:
                                late_input_wave(1)
                            if kt < NKT - 2:
                                v_chain(kt + 2)
                            if (kt + 3) % 4 == 0 and (kt + 3) // 4 < NQB:
                                k_chain((kt + 3) // 4)
                        if kt == 2 and qb + 1 < NQB:
                            q_chain(qb + 1)
                        if kt == 6 and qb > 0:
                            proj_block(qb - 1)
                    gg = g - 3
                    if gg >= 0:
                        qb2, kt2 = divmod(gg, NKT)
                        pv_step(qb2, kt2)
                        if kt2 == NKT - 1:
                            qb_finish(qb2)
                proj_block(NQB - 1, tail=True)


def build_nc():
    import concourse.mybir as mybir
    import concourse.bass as bass
    import concourse.tile as tile
    from concourse import bacc

    f32 = mybir.dt.float32
    f16 = mybir.dt.float16
    nc = bacc.Bacc("TRN2", target_bir_lowering=False, debug=False)
    shapes = {
        "xT": ([C, L], f16),
        "wqkT": ([C, 2 * HPC * HD], f16),
        "wvT": ([C, HPC * HD], f16),
        "qkb": ([128, 2], f32),
        "vb": ([128, HPC * HD], f32),
        "cos2": ([128, L], f16),
        "sin2": ([128, L], f16),
        "prhT": ([128, 128], f16),
        "wpT": ([128, C], f16),
    }
    ins = {
        name: nc.dram_tensor(name, shp, dt, kind="ExternalInput").ap()
        for name, (shp, dt) in shapes.items()
    }
    out_ap = nc.dram_tensor("out", [L, C], f32, kind="ExternalOutput").ap()
    with tile.TileContext(nc) as tc:
        _emit(tc, nc, ins, out_ap, mybir, bass)
    nc.compile()
    return nc


def _rope_tables():
    """cos/sin tables, computed exactly like reference.rope_cos_sin (f32 jax on CPU)."""
    if "rope" in _NC_CACHE:
        return _NC_CACHE["rope"]
    import jax
    import jax.numpy as jnp

    with jax.default_device(jax.devices("cpu")[0]):
        idx = jnp.arange(0, HD, 2, dtype=jnp.float32)
        inv_freq = 1.0 / 10000.0 ** (idx / HD)
        t = jnp.arange(L, dtype=jnp.float32)
        freqs = t[:, None] * inv_freq[None, :]
        emb = jnp.concatenate([freqs, freqs], axis=-1)  # (L, hd)
        cos = np.asarray(jnp.cos(emb), dtype=np.float32)
        sin = np.asarray(jnp.sin(emb), dtype=np.float32)
    _NC_CACHE["rope"] = (cos, sin)
    return cos, sin


def host_inputs(x, qkv_w, qkv_b, proj_w, core):
    b, g = core // GROUPS, core % GROUPS
    h0 = HPC * g
    fsl = slice(h0 * HD, (h0 + HPC) * HD)       # this core's 128 feature rows
    cos, sin = _rope_tables()
    cosT = np.ascontiguousarray(cos.T)           # [hd, L]
    sinT = np.ascontiguousarray(sin.T)

    wq = qkv_w[0 * C:1 * C][fsl]                 # [128, C]
    wk = qkv_w[1 * C:2 * C][fsl]
    wv = qkv_w[2 * C:3 * C][fsl]
    bq = qkv_b[0 * C:1 * C][fsl]
    bk = qkv_b[1 * C:2 * C][fsl]
    bv = qkv_b[2 * C:3 * C][fsl]

    prhT = np.zeros((128, 128), np.float32)
    for hh in (0, HD):
        for i in range(HD // 2):
            prhT[hh + 2 * i + 1, hh + 2 * i] = -1.0   # rh[2i] = -q[2i+1]
            prhT[hh + 2 * i, hh + 2 * i + 1] = 1.0    # rh[2i+1] = q[2i]

    wpT = np.concatenate(
        [np.ascontiguousarray(proj_w[:, (h0 + j) * HD:(h0 + j + 1) * HD].T) for j in range(HPC)],
        axis=0,
    )  # [128, C]: rows 0-63 head0, 64-127 head1

    return {
        "xT": np.ascontiguousarray(x[b].T).astype(np.float16),
        "wqkT": np.ascontiguousarray(np.concatenate([wq, wk], 0).T).astype(np.float16),
        "wvT": np.ascontiguousarray(wv.T).astype(np.float16),
        "qkb": np.ascontiguousarray(np.stack([bq, bk], 1)),
        "vb": np.broadcast_to(bv[None, :], (128, HPC * HD)).copy(),
        "cos2": np.concatenate([cosT, cosT], 0).astype(np.float16),
        "sin2": np.concatenate([sinT, sinT], 0).astype(np.float16),
        "prhT": prhT.astype(np.float16),
        "wpT": wpT.astype(np.float16),
    }


def kernel(x, qkv_w, qkv_b, proj_w, proj_b, _trace=False):
    from concourse.bass_utils import run_bass_kernel_spmd

    x = np.asarray(x, np.float32)
    qkv_w = np.asarray(qkv_w, np.float32)
    qkv_b = np.asarray(qkv_b, np.float32)
    proj_w = np.asarray(proj_w, np.float32)
    proj_b = np.asarray(proj_b, np.float32)

    if "nc" not in _NC_CACHE:
        _NC_CACHE["nc"] = build_nc()
    nc = _NC_CACHE["nc"]
    in_maps = [host_inputs(x, qkv_w, qkv_b, proj_w, c) for c in range(NCORES)]
    res = None
    last_err = None
    for attempt in range(3):
        try:
            res = run_bass_kernel_spmd(
                nc, in_maps, core_ids=list(range(NCORES)), trace=_trace
            )
            break
        except Exception as e:  # transient NRT device errors recover on retry
            last_err = e
            import time as _time
            _time.sleep(2.0)
    if res is None:
        raise last_err
    out = np.zeros((B, L, C), np.float32)
    for c in range(NCORES):
        out[c // GROUPS] += res.results[c]["out"]
    out += proj_b[None, None, :]
    if _trace:
        _NC_CACHE["last_results"] = res
    return out


# revision 30
# speedup vs baseline: 1.1507x; 1.1507x over previous
"""Trainium2 Bass kernel for nn_Attention_42253888258536.

Full-precision (fp32) multi-head attention with RoPE:
  qkv = x @ qkv_w.T + qkv_b ; RoPE(q, k) ; softmax(q k^T / sqrt(hd)) @ v ; proj.

Sharding: 8 cores = 2 batches x 4 head-groups (2 heads each). Each core
computes its heads' attention and a partial output projection (row-parallel
over proj_w columns); the host sums 4 partials per batch and adds proj_b.

Per-core device pipeline (all fp32 accumulation):
  1. q^T/k^T = W @ x^T via PE (weights stationary), v in natural layout.
  2. RoPE in transposed layout: rotate_half as a permutation matmul on PE,
     combine with cos/sin tables on DVE (f16 temporaries for 2x DVE modes).
  3. Attention over S^T = k_rot q_rot^T tiles: the two heads' K=64 matmuls
     run concurrently as PE row-tiles; exp on ACT (scale=1/8, bias=-5 fused).
     The ACT engine is the roofline for this kernel (1 elem/lane/cycle
     @1.2GHz over B*H*L^2/8 = 33.5M scores per core), so a tunable fraction
     of the exp tiles is offloaded to the otherwise-idle DVE using a
     Schraudolph-style bit trick: bits16 = (int16)(A*s + B) reinterpreted
     as bfloat16 gives exp(s/8-5) to ~1.8% rel, which softmax normalization
     largely cancels (<=1.3e-2 end-to-end vs the 2e-2 gate).
  4. P@V accumulated in PSUM with a ones-row appended to V so the softmax
     denominator Z falls out of the same matmul (row 64 of the ctx copy).
  5. Deferred normalization: out_h = (ctx_h @ Wp_h^T) * (1/Z) per partition,
     heads combined on DVE, partial written to DRAM.
"""

import sys

sys.path.insert(0, "/opt/trn_rl_repo")

import numpy as np

B, L, C = 2, 4096, 512
H, HD = 8, 64
NCORES = 8
HPC = 2          # heads per core
GROUPS = 4       # head groups (cores per batch)
QB = 512         # q-block (columns per S^T matmul)
NQB = L // QB    # 8
KT = 128         # k-tile (partitions per S^T tile)
NKT = L // KT    # 32

# DVE exp-offload (Schraudolph bits-as-bf16). OFF_MOD=0 disables.
OFF_MOD = 4
LOG2E = 1.4426950408889634
SCH_A = 128.0 * LOG2E / 8.0                       # includes the 1/8 score scale
SCH_SIGMA = 7.0                                   # mean-centering offset
SCH_B = 128.0 * (127.0 - 5.0 * LOG2E) - SCH_SIGMA

_NC_CACHE = {}


def _offloaded(qb, kt):
    # Split tiles: ACT exps head0's half (fits the ring bubble the full
    # offload would create), DVE bit-tricks head1's half. qb0's early kts
    # are skipped (DVE busy with k/v chains there).
    return OFF_MOD > 0 and (qb >= 1 or kt >= 8) and kt % 2 == 1


def _emit(tc, nc, ins, out_ap, mybir, bass):
    f32 = mybir.dt.float32
    f16 = mybir.dt.float16           # full-rate PE dtype that also keeps the HAM clock gate warm
    bf16 = mybir.dt.bfloat16
    i16 = mybir.dt.int16
    Exp = mybir.ActivationFunctionType.Exp
    Alu = mybir.AluOpType

    xT, wqkT, wvT, qkb, vb, cos2, sin2, prhT, wpT = (
        ins["xT"], ins["wqkT"], ins["wvT"], ins["qkb"], ins["vb"],
        ins["cos2"], ins["sin2"], ins["prhT"], ins["wpT"],
    )

    with tc.tile_pool(name="const", bufs=1) as const:
        xT_sb = const.tile([128, 4, L], f16)
        wqk_sb = const.tile([128, 4, 2 * HPC * HD], f16)
        wv_sb = const.tile([128, 4, HPC * HD], f16)
        qkb_sb = const.tile([128, 2], f32)
        vb_sb = const.tile([128, HPC * HD], f32)
        cos_sb = const.tile([128, L], f16)
        sin_sb = const.tile([128, L], f16)
        prh_sb = const.tile([128, 128], f16)
        wp_sb = const.tile([128, C], f16)
        expbias = const.tile([128, 1], f32)
        actwarm = const.tile([128, 1], f32)
        nc.vector.memset(expbias[:], -5.0)

        # DMA priority order: stream inputs in the order the qb=0 pipeline
        # consumes them (block 0 weights/x first, then x block-by-block with
        # its cos/sin). The scalar queue only carries kickoffs that complete
        # before the first ACTIVATE; everything else is sync/gpsimd (the ACT
        # engine is the roofline and every queued kickoff costs ~620ns).
        # Only the sync and scalar queues feed fast HWDGE rings (~90 GB/s
        # each); the gpsimd SWDGE ring is ~17 GB/s, so it carries no bulk
        # input. k_chain(0) consumes (xT b0 cc, wqk cc) in cc order:
        # interleave so its accumulation starts as soon as chunks land.
        for cc in range(4):
            eng = nc.sync if cc % 2 == 0 else nc.scalar
            eng.dma_start(xT_sb[:, cc, 0:QB], xT[cc * 128:(cc + 1) * 128, 0:QB])
            eng.dma_start(wqk_sb[:, cc, :], wqkT[cc * 128:(cc + 1) * 128, :])
        nc.sync.dma_start(qkb_sb[:], qkb[:])
        nc.scalar.dma_start(prh_sb[:], prhT[:])
        nc.sync.dma_start(cos_sb[:, 0:QB], cos2[:, 0:QB])
        nc.scalar.dma_start(sin_sb[:, 0:QB], sin2[:, 0:QB])
        # x block 1 plus its cos/sin (q_chain(1) fires at qb0/kt==2)
        for cc in range(4):
            eng = nc.sync if cc % 2 == 0 else nc.scalar
            eng.dma_start(xT_sb[:, cc, QB:2 * QB], xT[cc * 128:(cc + 1) * 128, QB:2 * QB])
        nc.sync.dma_start(cos_sb[:, QB:2 * QB], cos2[:, QB:2 * QB])
        nc.scalar.dma_start(sin_sb[:, QB:2 * QB], sin2[:, QB:2 * QB])
        # dummy exp: forces the ~2.7us exp ACT_TABLE_LOAD to overlap the
        # input DMAs instead of stalling the first real exp
        nc.scalar.activation(actwarm[:], expbias[:], Exp, bias=expbias[:], scale=1.0)
        for cc in range(4):
            nc.sync.dma_start(wv_sb[:, cc, :], wvT[cc * 128:(cc + 1) * 128, :])
        nc.sync.dma_start(vb_sb[:], vb[:])
        # x blocks 2-7 in two 1536-col waves per channel group; the sync ring
        # takes cc0/2 + cos, the scalar ring's share is kicked off from inside
        # the qb0 loop (between early exps) to stay behind the warm activation.
        for wv_ in range(2):
            wsl = slice((2 + 3 * wv_) * QB, (5 + 3 * wv_) * QB)
            for cc in (0, 2):
                nc.sync.dma_start(xT_sb[:, cc, wsl], xT[cc * 128:(cc + 1) * 128, wsl])
            nc.sync.dma_start(cos_sb[:, wsl], cos2[:, wsl])
        nc.sync.dma_start(wp_sb[:], wpT[:])

        def late_input_wave(wv_):
            wsl = slice((2 + 3 * wv_) * QB, (5 + 3 * wv_) * QB)
            for cc in (1, 3):
                nc.scalar.dma_start(xT_sb[:, cc, wsl], xT[cc * 128:(cc + 1) * 128, wsl])
            nc.scalar.dma_start(sin_sb[:, wsl], sin2[:, wsl])

        with tc.tile_pool(name="work", bufs=1) as work:
            qT_sb = work.tile([128, L], f16)   # 2 heads x 64 dims on partitions
            kT_sb = work.tile([128, L], f16)
            # v_aug[:, kt, 65*h : 65*h+65] = [V_h | ones] for k-tile kt
            v_aug = work.tile([128, NKT, 2 * (HD + 1)], f16)
            # ctxB rows 0-63 head0 ctx, row 64 holds Z0 until the zc gather,
            # rows 64-127 head1 ctx after the ctx1s DMA; ctx1s row 64 = Z1.
            ctxB = work.tile([128, L], f16)
            ctx1s = work.tile([HD + 1, L], f16)

            nc.vector.memset(v_aug[:, :, HD:HD + 1], 1.0)
            nc.vector.memset(v_aug[:, :, 2 * HD + 1:2 * HD + 2], 1.0)

            # ---- attention with fused v/q/proj pipelines ----
            with tc.tile_pool(name="spsum", bufs=2, space="PSUM") as spsum, \
                 tc.tile_pool(name="pv0ps", bufs=1, space="PSUM") as pv0ps, \
                 tc.tile_pool(name="pv1ps", bufs=1, space="PSUM") as pv1ps, \
                 tc.tile_pool(name="auxps", bufs=1, space="PSUM") as auxps, \
                 tc.tile_pool(name="psb", bufs=7) as psb, \
                 tc.tile_pool(name="psb16", bufs=5) as psb16, \
                 tc.tile_pool(name="auxsb", bufs=4) as auxsb, \
                 tc.tile_pool(name="zsb", bufs=10) as zsb, \
                 tc.tile_pool(name="outsb", bufs=3) as outsb:

                def k_chain(lb):
                    # k projection + RoPE for one 512-block on the aux banks;
                    # blocks 1-7 are emitted inside qb=0's kt loop so attention
                    # starts as soon as k-block 0 is roped.
                    lsl = bass.ts(lb, QB)
                    ps = auxps.tile([128, QB], f32, tag="aux0", name="kps")[:]
                    for cc in range(4):
                        nc.tensor.matmul(ps, wqk_sb[:, cc, 128:256], xT_sb[:, cc, lsl],
                                         start=(cc == 0), stop=(cc == 3))
                    nc.vector.tensor_scalar_add(kT_sb[:, lsl], ps, qkb_sb[:, 1:2])
                    rh = auxps.tile([128, QB], f32, tag="aux1", name="krh")[:]
                    nc.tensor.matmul(rh, prh_sb[:], kT_sb[:, lsl], start=True, stop=True)
                    t1 = auxsb.tile([128, QB], f16, tag="qt1")
                    nc.vector.tensor_mul(t1[:], kT_sb[:, lsl], cos_sb[:, lsl])
                    t2 = auxsb.tile([128, QB], f16, tag="qt2")
                    nc.vector.tensor_mul(t2[:], rh, sin_sb[:, lsl])
                    nc.vector.tensor_add(kT_sb[:, lsl], t1[:], t2[:])

                def q_chain(lb):
                    # q projection + RoPE for one 512-block, time-sharing the
                    # two aux PSUM banks with the projection pipeline.
                    lsl = bass.ts(lb, QB)
                    ps = auxps.tile([128, QB], f32, tag="aux0", name="qps")[:]
                    for cc in range(4):
                        nc.tensor.matmul(ps, wqk_sb[:, cc, 0:128], xT_sb[:, cc, lsl],
                                         start=(cc == 0), stop=(cc == 3))
                    nc.vector.tensor_scalar_add(qT_sb[:, lsl], ps, qkb_sb[:, 0:1])
                    rh = auxps.tile([128, QB], f32, tag="aux1", name="qrh")[:]
                    nc.tensor.matmul(rh, prh_sb[:], qT_sb[:, lsl], start=True, stop=True)
                    t1 = auxsb.tile([128, QB], f16, tag="qt1")
                    nc.vector.tensor_mul(t1[:], qT_sb[:, lsl], cos_sb[:, lsl])
                    t2 = auxsb.tile([128, QB], f16, tag="qt2")
                    nc.vector.tensor_mul(t2[:], rh, sin_sb[:, lsl])
                    nc.vector.tensor_add(qT_sb[:, lsl], t1[:], t2[:])

                def v_chain(lt):
                    ps = auxps.tile([128, 128], f32, tag="aux1", name="vps")[:]
                    for cc in range(4):
                        nc.tensor.matmul(ps, xT_sb[:, cc, bass.ts(lt, 128)], wv_sb[:, cc, :],
                                         start=(cc == 0), stop=(cc == 3))
                    nc.vector.tensor_tensor(
                        v_aug[:, lt, :].rearrange("p (h x) -> p h x", h=2)[:, :, 0:HD],
                        ps.rearrange("p (h x) -> p h x", h=2),
                        vb_sb[:].rearrange("p (h x) -> p h x", h=2),
                        op=Alu.add,
                    )

                z_tiles = {}
                pv_state = {}
                p_tiles = {}

                def proj_block(qb, tail=False):
                    # projection + 1/Z + head-combine + output DMA for q-block qb.
                    # In the tail (last qb), the scalar queue is idle, so the
                    # head1 scaling moves to ACT and the serialized DVE chain
                    # roughly halves.
                    zt = z_tiles.pop(qb)
                    for j in range(QB // 128):
                        qi = qb * (QB // 128) + j
                        qisl = bass.ts(qi, 128)
                        zc = zsb.tile([128, 2], f32, tag="zc")
                        nc.vector.reciprocal(zc[:], zt[:, :, j])
                        p0 = auxps.tile([128, C], f32, tag="aux0", name="p0")
                        p1 = auxps.tile([128, C], f32, tag="aux1", name="p1")
                        nc.tensor.matmul(p0[:], ctxB[0:HD, qisl], wp_sb[0:HD, :],
                                         start=True, stop=True)
                        nc.tensor.matmul(p1[:], ctxB[HD:128, qisl], wp_sb[HD:128, :],
                                         start=True, stop=True)
                        tmp = auxsb.tile([128, C], f32, tag="tmp")
                        if tail:
                            nc.scalar.activation(tmp[:], p1[:],
                                                 mybir.ActivationFunctionType.Copy,
                                                 scale=zc[:, 1:2])
                        else:
                            nc.vector.tensor_scalar_mul(tmp[:], p1[:], zc[:, 1:2])
                        ot = outsb.tile([128, C], f32, tag="ot")
                        nc.vector.scalar_tensor_tensor(
                            ot[:], p0[:], zc[:, 0:1], tmp[:],
                            op0=Alu.mult, op1=Alu.add,
                        )
                        eng = nc.scalar if tail and j % 2 else nc.sync
                        eng.dma_start(out_ap[qisl, :], ot[:])

                def pv_step(qb, kt):
                    pv0, pv1 = pv_state[qb]
                    off, pk = p_tiles.pop(qb * NKT + kt)
                    if pk is None:
                        pf, pi = off
                        r0 = pf[:, 0, :]
                        r1 = pi[:, 1, :].bitcast(bf16)
                    else:
                        r0 = pk[:, 0, :].bitcast(bf16) if off else pk[:, 0, :]
                        r1 = pk[:, 1, :].bitcast(bf16) if off else pk[:, 1, :]
                    nc.tensor.matmul(pv0[:], v_aug[:, kt, 0:HD + 1], r0,
                                     start=(kt == 0), stop=(kt == NKT - 1),
                                     skip_group_check=True)
                    nc.tensor.matmul(pv1[:], v_aug[:, kt, HD + 1:2 * (HD + 1)],
                                     r1,
                                     start=(kt == 0), stop=(kt == NKT - 1),
                                     skip_group_check=True)

                def qb_finish(qb):
                    # ctx copies (z rides in row 64), batched z gathers, then the
                    # head1 ctx DMA (which overwrites ctxB row 64 -- gathers first).
                    qsl = bass.ts(qb, QB)
                    pv0, pv1 = pv_state.pop(qb)
                    nc.vector.tensor_copy(ctxB[0:HD + 1, qsl], pv0[0:HD + 1, :])
                    nc.vector.tensor_copy(ctx1s[:, qsl], pv1[0:HD + 1, :])
                    zt = zsb.tile([128, 2, 4], f16, tag="zt")
                    for j in range(QB // 128):
                        qisl = bass.ts(qb * (QB // 128) + j, 128)
                        geng = nc.sync if j % 2 == 0 else \
                            (nc.scalar if qb == NQB - 1 else nc.sync)
                        geng.dma_start(zt[:, 0, j:j + 1], ctxB[HD:HD + 1, qisl])
                        geng.dma_start(zt[:, 1, j:j + 1], ctx1s[HD:HD + 1, qisl])
                    z_tiles[qb] = zt
                    nc.sync.dma_start(ctxB[HD:128, qsl], ctx1s[0:HD, qsl])

                # only k-block 0 and q-block 0 gate the first S matmul;
                # the remaining k/v-chains stream in during the first kt's,
                # just-in-time so a chain stalled on an input DMA never
                # head-of-line blocks a ready S matmul on the PE queue.
                # The PV/proj consumption runs 3 steps behind the S/exp stream
                # ACROSS qb boundaries, so the next block's first S never waits
                # behind the previous block's PV tail on the PE queue.
                k_chain(0)
                q_chain(0)
                v_chain(0)
                v_chain(1)
                TOT = NQB * NKT
                for g in range(TOT + 3):
                    if g < TOT:
                        qb, kt = divmod(g, NKT)
                        qsl = bass.ts(qb, QB)
                        if kt == 0:
                            pv_state[qb] = (
                                pv0ps.tile([HD + 1, QB], f32, tag="pv0", name="pv0"),
                                pv1ps.tile([HD + 1, QB], f32, tag="pv1", name="pv1"),
                            )
                        ksl = bass.ts(kt, KT)
                        s = spsum.tile([128, 2, QB], f32, tag="s")
                        nc.tensor.matmul(s[:, 0, :], kT_sb[0:HD, ksl],
                                         qT_sb[0:HD, qsl], start=True, stop=True)
                        nc.tensor.matmul(s[:, 1, :], kT_sb[HD:128, ksl],
                                         qT_sb[HD:128, qsl], start=True, stop=True)
                        if _offloaded(qb, kt):
                            # Split consumption: ACT computes exact exp on the
                            # head0 half; the DVE computes the head1 half via
                            # Schraudolph ((int16)(A*s+B) bits == bf16
                            # exp(s/8-5)). High priority on the DVE half: it
                            # gates the 2-deep S-tile ring.
                            pf = psb.tile([128, 2, QB], f16, tag="p", name="pf")
                            nc.scalar.activation(pf[:, 0, :], s[:, 0, :], Exp,
                                                 bias=expbias[:], scale=0.125)
                            pi = psb16.tile([128, 2, QB], i16, tag="poff", name="pi")
                            with tc.high_priority():
                                nc.vector.tensor_scalar(
                                    pi[:, 1, :], s[:, 1, :], SCH_A, SCH_B,
                                    op0=Alu.mult, op1=Alu.add,
                                )
                            p_tiles[g] = ((pf, pi), None)
                        else:
                            # exp(s/8 - 5): the shift keeps the f16 exp output far
                            # from overflow; softmax is shift-invariant since Z
                            # accumulates the same e^-5.
                            p = psb.tile([128, 2, QB], f16, tag="p", name="p")
                            nc.scalar.activation(p[:], s[:], Exp, bias=expbias[:],
                                                 scale=0.125)
                            p_tiles[g] = (False, p)
                        if qb == 0:
                            if kt == 0:
                                late_input_wave(0)
                            elif kt == 2:
                                late_input_wave(1)
                            if kt < NKT - 2:
                                v_chain(kt + 2)
                            if (kt + 3) % 4 == 0 and (kt + 3) // 4 < NQB:
                                k_chain((kt + 3) // 4)
                        if kt == 2 and qb + 1 < NQB:
                            q_chain(qb + 1)
                        if kt == 6 and qb > 0:
                            proj_block(qb - 1)
                    gg = g - 3
                    if gg >= 0:
                        qb2, kt2 = divmod(gg, NKT)
                        pv_step(qb2, kt2)
                        if kt2 == NKT - 1:
                            qb_finish(qb2)
                proj_block(NQB - 1, tail=True)


def build_nc():
    import concourse.mybir as mybir
    import concourse.bass as bass
    import concourse.tile as tile
    from concourse import bacc

    f32 = mybir.dt.float32
    f16 = mybir.dt.float16
    nc = bacc.Bacc("TRN2", target_bir_lowering=False, debug=False)
    shapes = {
        "xT": ([C, L], f16),
        "wqkT": ([C, 2 * HPC * HD], f16),
        "wvT": ([C, HPC * HD], f16),
        "qkb": ([128, 2], f32),
        "vb": ([128, HPC * HD], f32),
        "cos2": ([128, L], f16),
        "sin2": ([128, L], f16),
        "prhT": ([128, 128], f16),
        "wpT": ([128, C], f16),
    }
    ins = {
        name: nc.dram_tensor(name, shp, dt, kind="ExternalInput").ap()
        for name, (shp, dt) in shapes.items()
    }
    out_ap = nc.dram_tensor("out", [L, C], f32, kind="ExternalOutput").ap()
    with tile.TileContext(nc) as tc:
        _emit(tc, nc, ins, out_ap, mybir, bass)
    nc.compile()
    return nc


def _rope_tables():
    """cos/sin tables, computed exactly like reference.rope_cos_sin (f32 jax on CPU)."""
    if "rope" in _NC_CACHE:
        return _NC_CACHE["rope"]
    import jax
    import jax.numpy as jnp

    with jax.default_device(jax.devices("cpu")[0]):
        idx = jnp.arange(0, HD, 2, dtype=jnp.float32)
        inv_freq = 1.0 / 10000.0 ** (idx / HD)
        t = jnp.arange(L, dtype=jnp.float32)
        freqs = t[:, None] * inv_freq[None, :]
        emb = jnp.concatenate([freqs, freqs], axis=-1)  # (L, hd)
        cos = np.asarray(jnp.cos(emb), dtype=np.float32)
        sin = np.asarray(jnp.sin(emb), dtype=np.float32)
    _NC_CACHE["rope"] = (cos, sin)
    return cos, sin


def host_inputs(x, qkv_w, qkv_b, proj_w, core):
    b, g = core // GROUPS, core % GROUPS
    h0 = HPC * g
    fsl = slice(h0 * HD, (h0 + HPC) * HD)       # this core's 128 feature rows
    cos, sin = _rope_tables()
    cosT = np.ascontiguousarray(cos.T)           # [hd, L]
    sinT = np.ascontiguousarray(sin.T)

    wq = qkv_w[0 * C:1 * C][fsl]                 # [128, C]
    wk = qkv_w[1 * C:2 * C][fsl]
    wv = qkv_w[2 * C:3 * C][fsl]
    bq = qkv_b[0 * C:1 * C][fsl]
    bk = qkv_b[1 * C:2 * C][fsl]
    bv = qkv_b[2 * C:3 * C][fsl]

    prhT = np.zeros((128, 128), np.float32)
    for hh in (0, HD):
        for i in range(HD // 2):
            prhT[hh + 2 * i + 1, hh + 2 * i] = -1.0   # rh[2i] = -q[2i+1]
            prhT[hh + 2 * i, hh + 2 * i + 1] = 1.0    # rh[2i+1] = q[2i]

    wpT = np.concatenate(
        [np.ascontiguousarray(proj_w[:, (h0 + j) * HD:(h0 + j + 1) * HD].T) for j in range(HPC)],
        axis=0,
    )  # [128, C]: rows 0-63 head0, 64-127 head1

    return {
        "xT": np.ascontiguousarray(x[b].T).astype(np.float16),
        "wqkT": np.ascontiguousarray(np.concatenate([wq, wk], 0).T).astype(np.float16),
        "wvT": np.ascontiguousarray(wv.T).astype(np.float16),
        "qkb": np.ascontiguousarray(np.stack([bq, bk], 1)),
        "vb": np.broadcast_to(bv[None, :], (128, HPC * HD)).copy(),
        "cos2": np.concatenate([cosT, cosT], 0).astype(np.float16),
        "sin2": np.concatenate([sinT, sinT], 0).astype(np.float16),
        "prhT": prhT.astype(np.float16),
        "wpT": wpT.astype(np.float16),
    }


def kernel(x, qkv_w, qkv_b, proj_w, proj_b, _trace=False):
    from concourse.bass_utils import run_bass_kernel_spmd

    x = np.asarray(x, np.float32)
    qkv_w = np.asarray(qkv_w, np.float32)
    qkv_b = np.asarray(qkv_b, np.float32)
    proj_w = np.asarray(proj_w, np.float32)
    proj_b = np.asarray(proj_b, np.float32)

    if "nc" not in _NC_CACHE:
        _NC_CACHE["nc"] = build_nc()
    nc = _NC_CACHE["nc"]
    in_maps = [host_inputs(x, qkv_w, qkv_b, proj_w, c) for c in range(NCORES)]
    res = None
    last_err = None
    for attempt in range(3):
        try:
            res = run_bass_kernel_spmd(
                nc, in_maps, core_ids=list(range(NCORES)), trace=_trace
            )
            break
        except Exception as e:  # transient NRT device errors recover on retry
            last_err = e
            import time as _time
            _time.sleep(2.0)
    if res is None:
        raise last_err
    out = np.zeros((B, L, C), np.float32)
    for c in range(NCORES):
        out[c // GROUPS] += res.results[c]["out"]
    out += proj_b[None, None, :]
    if _trace:
        _NC_CACHE["last_results"] = res
    return out
